# revision 1
# baseline (speedup 1.0000x reference)
"""Multi-head graph attention (GAT-style message passing) on 8 Trainium2 cores.

Math (per head i, diag transform):
    h        = x * w[i]                      # [N, d]
    p_src    = h @ a[:d],  p_dst = h @ a[d:] # [N]
    s_e      = p_src[src_e] + p_dst[dst_e]   # per edge
    e_e      = exp(-leaky_relu(s_e, 0.2))
    out[i,n] = (sum_{e: src=n} e_e * h[dst_e]) / (sum_{e: src=n} e_e)

Key identity: w[i] (a per-channel diagonal) commutes with the segment sum, so
    out[i,n] = w[i] * (sum e_e * x[dst_e]) / rowsum_n
and the expensive gather of x[dst] is shared by all 4 heads.

Strategy:
  - Host: fold (w, attn) -> A [d, 2H]; sort edges by src; partition nodes
    equally across 8 cores; pack each core's edges into 128-edge tiles that
    never split a node and span <= 16 nodes; groups of 32 tiles.
  - Launch 1 (tiny): P = x @ A computed distributed ([N,8] fp32), host concats.
  - Launch 2 (main, per core): indirect-DMA gather x16[dst] (fp16 rows) and
    P[src], P[dst]; scores -> e on ACT; one-hot matrices on DVE from
    host-provided local offsets; TensorE does the segment sum into PSUM
    (128-edge x 16-node windows, statically placed); rowsums via e-stationary
    matmuls; reciprocal + scale; output written [c-major], host transposes.
"""

import os

import numpy as np

from concourse import bacc, bass, mybir
import concourse.tile as tile
from concourse.bass import IndirectOffsetOnAxis
from concourse.bass_utils import run_bass_kernel_spmd

# test.py sets GAT_TRACE=1 to profile; results of the last kernel() call are
# stashed here so the harness can report HW time.
LAST_RESULTS = []

F32 = mybir.dt.float32
F16 = mybir.dt.float16
I32 = mybir.dt.int32

N_CORES = 8
TILE_E = 128      # edges per tile (partition dim)
W = 16            # max node span of a tile (one-hot width)
TPG = 32          # tiles per group (=> 512 node-slots per group, one PSUM bank row)
EPG = TILE_E * TPG  # edges per group


# --------------------------------------------------------------------------
# host-side layout preprocessing
# --------------------------------------------------------------------------

def _pack_core(src, dst, n_lo, n_hi):
    """Pack one core's (sorted-by-src) edges into tiles.

    Returns (tiles, spans) where tiles is a list of (base_node, edge_idx_array)
    and spans[t] = number of nodes covered by tile t.
    Guarantees: a node's edges are never split across tiles; span <= W;
    <= TILE_E edges per tile.
    """
    # edge index range for this core
    lo = np.searchsorted(src, n_lo, side="left")
    hi = np.searchsorted(src, n_hi, side="left")
    s = src[lo:hi]
    # per-node edge counts within [n_lo, n_hi)
    counts = np.bincount(s - n_lo, minlength=n_hi - n_lo)
    assert counts.max() <= TILE_E, "node degree exceeds one tile"
    starts = lo + np.concatenate([[0], np.cumsum(counts)[:-1]])

    tiles = []
    spans = []
    cur_edges = 0
    cur_base = None
    cur_start = None
    cur_nodes = 0
    for ni in range(n_hi - n_lo):
        c = int(counts[ni])
        node = n_lo + ni
        if cur_base is None:
            cur_base, cur_start, cur_edges, cur_nodes = node, int(starts[ni]), c, 1
            continue
        if cur_edges + c > TILE_E or (node - cur_base) >= W:
            tiles.append((cur_base, cur_start, cur_edges))
            spans.append(cur_nodes)
            cur_base, cur_start, cur_edges, cur_nodes = node, int(starts[ni]), c, 1
        else:
            cur_edges += c
            cur_nodes = node - cur_base + 1
    if cur_base is not None:
        tiles.append((cur_base, cur_start, cur_edges))
        spans.append(cur_nodes)
    return tiles, spans


def _prep_edges(src, dst, n_nodes):
    """Sort by src, partition nodes across cores, build per-core tile arrays.

    Returns dict with per-core arrays (lists of length N_CORES) and G.
    """
    order = np.argsort(src, kind="stable")
    src_s = src[order].astype(np.int32)
    dst_s = dst[order].astype(np.int32)

    npc = n_nodes // N_CORES
    per_core = []
    for c in range(N_CORES):
        n_lo, n_hi = c * npc, (c + 1) * npc if c < N_CORES - 1 else n_nodes
        tiles, spans = _pack_core(src_s, dst_s, n_lo, n_hi)
        per_core.append((tiles, spans))

    G = max((len(t[0]) + TPG - 1) // TPG for t in per_core)

    dsti, srci, loc, colmap = [], [], [], []
    for c in range(N_CORES):
        tiles, spans = per_core[c]
        nt = G * TPG
        d_idx = np.zeros((nt, TILE_E), dtype=np.int32)
        s_idx = np.zeros((nt, TILE_E), dtype=np.int32)
        l_arr = np.full((nt, TILE_E), -1.0, dtype=np.float16)
        cmap = np.full((nt, W), -1, dtype=np.int64)
        for t, (base, estart, ecnt) in enumerate(tiles):
            d_idx[t, :ecnt] = dst_s[estart:estart + ecnt]
            s_idx[t, :ecnt] = src_s[estart:estart + ecnt]
            l_arr[t, :ecnt] = (src_s[estart:estart + ecnt] - base).astype(np.float16)
            cmap[t, :spans[t]] = np.arange(base, base + spans[t])
        # reshape to [G, 128, TPG]: tile t of group g at [:, t], edge p on partition p
        d4 = d_idx.reshape(G, TPG, TILE_E).transpose(0, 2, 1).copy()
        s4 = s_idx.reshape(G, TPG, TILE_E).transpose(0, 2, 1).copy()
        l4 = l_arr.reshape(G, TPG, TILE_E).transpose(0, 2, 1).copy()
        dsti.append(d4)
        srci.append(s4)
        loc.append(l4)
        colmap.append(cmap.reshape(G * TPG * W))
    return dict(dsti=dsti, srci=srci, loc=loc, colmap=colmap, G=G)


# --------------------------------------------------------------------------
# launch 1: P = x @ A   (distributed over node slabs)
# --------------------------------------------------------------------------

def _build_l1(nt):
    """xt: [128, nt*128] f32 (= x-slab transposed), amat: [128, 8] f32
    -> pout: [nt*128, 8] f32"""
    nc = bacc.Bacc(None)
    xt = nc.declare_dram_parameter("xt", [128, nt * 128], F32, isOutput=False)
    amat = nc.declare_dram_parameter("amat", [128, 8], F32, isOutput=False)
    pout = nc.declare_dram_parameter("pout", [nt * 128, 8], F32, isOutput=True)

    with tile.TileContext(nc) as tc:
        with (
            tc.tile_pool(name="sb", bufs=3) as sb,
            tc.tile_pool(name="cst", bufs=1) as cst,
            tc.tile_pool(name="ps", bufs=2, space="PSUM") as ps,
        ):
            a_sb = cst.tile([128, 8], F32)
            nc.sync.dma_start(out=a_sb[:], in_=amat[:, :])
            # walrus only allows one sync wait on a Matmult(LDW); this tiny
            # carrier matmul absorbs the a_sb DMA wait so the real matmuls
            # each carry a single xt-tile wait.
            dummy_ps = ps.tile([1, 1], F32, tag="dummy")
            nc.tensor.matmul(out=dummy_ps[:], lhsT=a_sb[:1, :1], rhs=a_sb[:1, :1],
                             start=True, stop=True)
            for t in range(nt):
                xt_sb = sb.tile([128, 128], F32, tag="xt")
                nc.sync.dma_start(out=xt_sb[:], in_=xt[:, t * 128:(t + 1) * 128])
                p_ps = ps.tile([128, 8], F32)
                nc.tensor.matmul(out=p_ps[:], lhsT=xt_sb[:], rhs=a_sb[:],
                                 start=True, stop=True)
                p_sb = sb.tile([128, 8], F32, tag="p")
                nc.vector.tensor_copy(out=p_sb[:], in_=p_ps[:])
                nc.sync.dma_start(out=pout[t * 128:(t + 1) * 128, :], in_=p_sb[:])
    nc.compile()
    return nc


# --------------------------------------------------------------------------
# launch 2: the main edge-parallel kernel
# --------------------------------------------------------------------------

def _build_l2(n_nodes, G):
    nc = bacc.Bacc(None)
    # t16 row n = [x16[n] (128) | P16[n] (8)] so one gather serves both the
    # feature row and p_dst
    t16 = nc.declare_dram_parameter("t16", [n_nodes, 136], F16, isOutput=False)
    ptab = nc.declare_dram_parameter("ptab", [n_nodes, 8], F32, isOutput=False)
    dsti = nc.declare_dram_parameter("dsti", [G, 128, TPG], I32, isOutput=False)
    colx = nc.declare_dram_parameter("colx", [G, 128, 4], I32, isOutput=False)
    locd = nc.declare_dram_parameter("locd", [G, 128, TPG], F16, isOutput=False)
    iotac = nc.declare_dram_parameter("iotac", [128, W], F16, isOutput=False)
    selc = nc.declare_dram_parameter("selc", [4, 512], F16, isOutput=False)
    sel32c = nc.declare_dram_parameter("sel32c", [32, 32 * 128], F16,
                                       isOutput=False)
    identc = nc.declare_dram_parameter("identc", [128, 128], F16, isOutput=False)
    wcol = nc.declare_dram_parameter("wcol", [128, 4], F32, isOutput=False)
    out = nc.declare_dram_parameter("out", [4, G, 128, TPG * W], F32, isOutput=True)

    with tile.TileContext(nc) as tc:
        with (
            tc.tile_pool(name="cst", bufs=1) as cst,
            tc.tile_pool(name="idx", bufs=3) as idxp,
            tc.tile_pool(name="gat", bufs=2) as gat,
            tc.tile_pool(name="mm", bufs=2) as mm,
            tc.tile_pool(name="epi", bufs=2) as epi,
            tc.tile_pool(name="outp", bufs=4) as outp,
            tc.tile_pool(name="ps", bufs=1, space="PSUM") as ps,
            tc.tile_pool(name="psb", bufs=1, space="PSUM") as psb,
            tc.tile_pool(name="pst", bufs=1, space="PSUM") as pst,
        ):
            iota_sb = cst.tile([128, W], F16)
            nc.sync.dma_start(out=iota_sb[:], in_=iotac[:, :])
            sel_sb = cst.tile([4, 512], F16)
            nc.sync.dma_start(out=sel_sb[:], in_=selc[:, :])
            sel32_sb = cst.tile([32, 32 * 128], F16)
            nc.sync.dma_start(out=sel32_sb[:], in_=sel32c[:, :])
            ident_sb = cst.tile([128, 128], F16)
            nc.sync.dma_start(out=ident_sb[:], in_=identc[:, :])
            w_sb = cst.tile([128, 4], F32)
            nc.sync.dma_start(out=w_sb[:], in_=wcol[:, :])

            for g in range(G):
                # ---- per-group metadata loads
                di = idxp.tile([128, TPG], I32, tag="di")
                ci = idxp.tile([128, 4], I32, tag="ci")
                lo = idxp.tile([128, TPG], F16, tag="lo")
                nc.sync.dma_start(out=di[:], in_=dsti[g, :, :])
                nc.sync.dma_start(out=ci[:], in_=colx[g, :, :])
                nc.sync.dma_start(out=lo[:], in_=locd[g, :, :])

                # ---- gathers (HW only honors one offset per partition, so
                # issue per-tile [128,1] indirect DMAs)
                xg = gat.tile([128, TPG, 136], F16, tag="xg")
                for t in range(TPG):
                    nc.gpsimd.indirect_dma_start(
                        out=xg[:, t, :], out_offset=None, in_=t16[:, :],
                        in_offset=IndirectOffsetOnAxis(ap=di[:, t:t + 1], axis=0))
                # P rows of this group's 512 column nodes (quarter q on
                # partition p = column q*128+p)
                pc = gat.tile([128, 4, 8], F32, tag="pc")
                for q in range(4):
                    nc.gpsimd.indirect_dma_start(
                        out=pc[:, q, :], out_offset=None, in_=ptab[:, :],
                        in_offset=IndirectOffsetOnAxis(ap=ci[:, q:q + 1], axis=0))

                # ---- scores, factored: with s = p_src[col] + p_dst[e],
                # exp(-lrelu(s)) = min(exp(-s), exp(-0.2 s))
                #               = min(A[col]*B[e], C[col]*D[e])
                # edge factors
                pd32 = mm.tile([128, TPG, 4], F32, tag="pd32")
                nc.vector.tensor_copy(out=pd32[:], in_=xg[:, :, 132:136])
                b16 = mm.tile([128, TPG, 4], F16, tag="b16")
                nc.scalar.activation(out=b16[:], in_=pd32[:],
                                     func=mybir.ActivationFunctionType.Exp,
                                     scale=-1.0)
                d16 = mm.tile([128, TPG, 4], F16, tag="d16")
                nc.scalar.activation(out=d16[:], in_=pd32[:],
                                     func=mybir.ActivationFunctionType.Exp,
                                     scale=-0.2)
                # column factors: acpack col j<16 -> A(q=j//4, i=j%4),
                # col 16+j -> C(q, i)
                acpack = mm.tile([128, 32], F16, tag="acpack")
                nc.scalar.activation(out=acpack[:, 0:16].rearrange(
                                         "p (q i) -> p q i", q=4, i=4),
                                     in_=pc[:, :, 0:4],
                                     func=mybir.ActivationFunctionType.Exp,
                                     scale=-1.0)
                nc.scalar.activation(out=acpack[:, 16:32].rearrange(
                                         "p (q i) -> p q i", q=4, i=4),
                                     in_=pc[:, :, 0:4],
                                     func=mybir.ActivationFunctionType.Exp,
                                     scale=-0.2)
                tp_ps = pst.tile([32, 128], F16, tag="tp")
                nc.tensor.transpose(out=tp_ps[:], in_=acpack[:],
                                    identity=ident_sb[:])
                act_sb = mm.tile([32, 128], F16, tag="act")
                nc.vector.tensor_copy(out=act_sb[:], in_=tp_ps[:])

                # ---- one-hot matrix
                m0 = mm.tile([128, TPG, W], F16, tag="m0")
                nc.vector.tensor_tensor(
                    out=m0[:],
                    in0=lo[:, :, None].broadcast_to([128, TPG, W]),
                    in1=iota_sb[:, None, :].broadcast_to([128, TPG, W]),
                    op=mybir.AluOpType.is_equal)

                # ---- per-head M' = min(A*U, C*V), U = M0*B, V = M0*D
                mall = mm.tile([128, TPG, 4, W], F16, tag="mall")
                e4 = mm.tile([128, TPG, 4], F16, tag="e4")
                for i in range(4):
                    bcpair = psb.tile([128, 2 * TPG * W], F32, tag="bcpair")
                    bca = bcpair[:, 0:TPG * W]
                    bcc = bcpair[:, TPG * W:2 * TPG * W]
                    for q in range(4):
                        ja = 4 * q + i
                        jc = 16 + 4 * q + i
                        nc.tensor.matmul(
                            out=bca[:, q * 128:(q + 1) * 128],
                            lhsT=sel32_sb[:, ja * 128:(ja + 1) * 128],
                            rhs=act_sb[:], start=True, stop=True)
                        nc.tensor.matmul(
                            out=bcc[:, q * 128:(q + 1) * 128],
                            lhsT=sel32_sb[:, jc * 128:(jc + 1) * 128],
                            rhs=act_sb[:], start=True, stop=True)
                    u16 = mm.tile([128, TPG, W], F16, tag="u16")
                    nc.vector.tensor_tensor(
                        out=u16[:], in0=m0[:],
                        in1=b16[:, :, i:i + 1].broadcast_to([128, TPG, W]),
                        op=mybir.AluOpType.mult)
                    v16 = mm.tile([128, TPG, W], F16, tag="v16")
                    nc.vector.tensor_tensor(
                        out=v16[:], in0=m0[:],
                        in1=d16[:, :, i:i + 1].broadcast_to([128, TPG, W]),
                        op=mybir.AluOpType.mult)
                    au = mm.tile([128, TPG, W], F32, tag="au")
                    nc.vector.tensor_tensor(
                        out=au[:], in0=u16[:],
                        in1=bca.rearrange("p (t w) -> p t w", t=TPG, w=W),
                        op=mybir.AluOpType.mult)
                    cv = mm.tile([128, TPG, W], F32, tag="cv")
                    nc.vector.tensor_tensor(
                        out=cv[:], in0=v16[:],
                        in1=bcc.rearrange("p (t w) -> p t w", t=TPG, w=W),
                        op=mybir.AluOpType.mult)
                    nc.vector.tensor_tensor(
                        out=mall[:, :, i, :],
                        in0=au[:], in1=cv[:], op=mybir.AluOpType.min)
                    # each M' row has exactly one nonzero -> no accumulation
                    with nc.allow_low_precision(reason="picking one nonzero"):
                        nc.vector.reduce_sum(
                            out=e4[:, :, i:i + 1],
                            in_=mall[:, :, i, :],
                            axis=mybir.AxisListType.X)

                # ---- segment sums on TensorE
                agg = ps.tile([128, TPG * 4 * W], F32, tag="agg")
                rs = ps.tile([4, TPG * W], F32, tag="rs")
                for t in range(TPG):
                    nc.tensor.matmul(
                        out=agg[:, t * 4 * W:(t + 1) * 4 * W],
                        lhsT=xg[:, t, 0:128], rhs=mall[:, t, :, :],
                        start=True, stop=True)
                    nc.tensor.matmul(
                        out=rs[:, t * W:(t + 1) * W],
                        lhsT=e4[:, t, :], rhs=m0[:, t, :],
                        start=True, stop=True)

                # ---- epilogue: out = w ⊙ agg / rowsum
                # clamp pad-column zeros so reciprocal stays finite (real
                # rowsums are >= exp(-|s|max) >> 3e-5)
                rsc = epi.tile([4, TPG * W], F32, tag="rsc")
                nc.vector.tensor_scalar(out=rsc[:], in0=rs[:], scalar1=3e-5,
                                        scalar2=None, op0=mybir.AluOpType.max)
                rsi32 = epi.tile([4, TPG * W], F32, tag="rsi32")
                nc.vector.reciprocal(out=rsi32[:], in_=rsc[:])
                rsi16 = epi.tile([4, TPG * W], F16, tag="rsi16")
                nc.vector.tensor_copy(out=rsi16[:], in_=rsi32[:])
                agg4 = agg[:].rearrange("p (t h w) -> p t h w", t=TPG, h=4, w=W)
                for i in range(4):
                    bc = psb.tile([128, TPG * W], F32, tag="bcpair")
                    nc.tensor.matmul(out=bc[:], lhsT=sel_sb[:, i * 128:(i + 1) * 128],
                                     rhs=rsi16[:], start=True, stop=True)
                    rinv = epi.tile([128, TPG * W], F32, tag="rinv")
                    nc.scalar.activation(out=rinv[:], in_=bc[:],
                                         func=mybir.ActivationFunctionType.Copy)
                    oh = outp.tile([128, TPG * W], F32, tag="oh")
                    oh4 = oh[:].rearrange("p (t w) -> p t w", t=TPG, w=W)
                    rinv4 = rinv[:].rearrange("p (t w) -> p t w", t=TPG, w=W)
                    nc.vector.scalar_tensor_tensor(
                        out=oh4, in0=agg4[:, :, i, :],
                        scalar=w_sb[:, i:i + 1],
                        in1=rinv4,
                        op0=mybir.AluOpType.mult, op1=mybir.AluOpType.mult)
                    nc.sync.dma_start(out=out[i, g, :, :], in_=oh[:])
    nc.compile()
    return nc


# --------------------------------------------------------------------------
# entry point
# --------------------------------------------------------------------------

def kernel(x, w, attn, edge):
    x = np.asarray(x, dtype=np.float32)
    w = np.asarray(w, dtype=np.float32)
    attn = np.asarray(attn, dtype=np.float32)
    edge = np.asarray(edge)

    n_nodes, d = x.shape
    n_heads = w.shape[0]
    assert d == 128 and n_heads == 4

    src = edge[0].astype(np.int64)
    dst = edge[1].astype(np.int64)

    # fold parameters: A[:, i] = w_i * a_src_i ; A[:, 4+i] = w_i * a_dst_i
    amat = np.zeros((128, 8), dtype=np.float32)
    for i in range(n_heads):
        amat[:, i] = w[i, 0, :] * attn[i, :d, 0]
        amat[:, 4 + i] = w[i, 0, :] * attn[i, d:, 0]

    # ---------------- launch 1: P = x @ A (node slabs)
    npc = n_nodes // N_CORES
    nt = (npc + 127) // 128
    nc1 = _build_l1(nt)
    in_maps1 = []
    for c in range(N_CORES):
        sl = x[c * npc:(c + 1) * npc]
        if sl.shape[0] < nt * 128:
            sl = np.concatenate(
                [sl, np.zeros((nt * 128 - sl.shape[0], d), np.float32)])
        in_maps1.append({"xt": np.ascontiguousarray(sl.T), "amat": amat})
    trace = bool(int(os.environ.get("GAT_TRACE", "0")))
    tkw = dict(trace=True, trace_cores=list(range(N_CORES))) if trace else {}

    def _run(nc, maps):
        try:
            return run_bass_kernel_spmd(nc, maps, list(range(N_CORES)), **tkw)
        except Exception:
            if not tkw:
                raise
            return run_bass_kernel_spmd(nc, maps, list(range(N_CORES)))

    r1 = _run(nc1, in_maps1)
    ptab = np.concatenate(
        [r1.results[c]["pout"][:npc] for c in range(N_CORES)], axis=0)
    ptab = np.ascontiguousarray(ptab[:n_nodes])

    # ---------------- host layout prep
    prep = _prep_edges(src, dst, n_nodes)
    G = prep["G"]

    # ---------------- launch 2
    nc2 = _build_l2(n_nodes, G)
    x16 = x.astype(np.float16)
    t16 = np.concatenate([x16, ptab.astype(np.float16)], axis=1)
    iota_c = np.broadcast_to(np.arange(W, dtype=np.float16), (128, W)).copy()
    sel_c = np.zeros((4, 512), dtype=np.float16)
    for i in range(4):
        sel_c[i, i * 128:(i + 1) * 128] = 1.0
    sel32_c = np.zeros((32, 32 * 128), dtype=np.float16)
    for j in range(32):
        sel32_c[j, j * 128:(j + 1) * 128] = 1.0
    ident_c = np.eye(128, dtype=np.float16)
    wcol = np.ascontiguousarray(w[:, 0, :].T)  # [128, 4]
    in_maps2 = []
    for c in range(N_CORES):
        cm = prep["colmap"][c].reshape(G, TPG * W)
        colx_c = np.maximum(cm, 0).astype(np.int32).reshape(
            G, 4, 128).transpose(0, 2, 1).copy()
        in_maps2.append({
            "t16": t16, "ptab": ptab,
            "dsti": prep["dsti"][c], "colx": colx_c,
            "locd": prep["loc"][c],
            "iotac": iota_c, "selc": sel_c, "sel32c": sel32_c,
            "identc": ident_c, "wcol": wcol,
        })
    r2 = _run(nc2, in_maps2)
    LAST_RESULTS.clear()
    LAST_RESULTS.extend([r1, r2])

    # ---------------- unshard: scatter tile-local columns to node rows
    out_full = np.zeros((n_heads, n_nodes, d), dtype=np.float32)
    for c in range(N_CORES):
        slab = r2.results[c]["out"]  # [4, G, 128, TPG*W]
        cm = prep["colmap"][c]       # [G*TPG*W] -> node or -1
        arr = slab.transpose(0, 1, 3, 2).reshape(n_heads, G * TPG * W, d)
        valid = cm >= 0
        out_full[:, cm[valid], :] = arr[:, valid, :]
    return out_full


if __name__ == "__main__":
    # smoke test with the real shapes is done via test.py
    pass



# revision 6
# speedup vs baseline: 1.3185x; 1.3185x over previous
"""Multi-head graph attention (GAT-style message passing) on 8 Trainium2 cores.

Math (per head i, diag transform):
    h        = x * w[i]                      # [N, d]
    p_src    = h @ a[:d],  p_dst = h @ a[d:] # [N]
    s_e      = p_src[src_e] + p_dst[dst_e]   # per edge
    e_e      = exp(-leaky_relu(s_e, 0.2))
    out[i,n] = (sum_{e: src=n} e_e * h[dst_e]) / (sum_{e: src=n} e_e)

Key identities:
  - w[i] commutes with the segment sum, so the x[dst] gather is shared by all
    4 heads and w[i] is applied at the very end.
  - exp(-leaky_relu(s)) = min(exp(-s), exp(-0.2 s)) and s factors into
    p_src[src] + p_dst[dst], so the per-edge weight is
    min(A[src]*B[dst], C[src]*D[dst]) with A,C per source column and B,D
    carried by the gathered destination row.

Layout ("quarter-window" scheme):
  - Edges are sorted by src and partitioned across 8 cores by src range.
  - A super-window = up to 32 consecutive src nodes whose per-dst-quarter
    edge counts each fit in 128.  Each window emits 4 chunks (one per dst
    quarter, 128 edge slots each); dst indices within a chunk are local to
    that quarter (< 25000), so they fit the int16 indices of the gpsimd
    dma_gather custom op.  One dma_gather per (group, quarter) replaces the
    128-row indirect DMAs that dominated the old kernel (994ns+ each on the
    Pool engine).
  - A group = 8 windows = 32 chunks = 4096 edge slots.  The 4 chunks of a
    window accumulate into the same PSUM block (matmul start/stop), so
    per-window rowsums and aggregates are complete on device.
  - p_src per edge slot comes from a host-built transposed one-hot (m0T)
    matmul against the window's 32 column P-values (gathered per group from
    a per-core P table, int16-local again).
  - Output is written f16, [4, G, 128, 8*32] per core, host scatters rows.
"""

import os

import numpy as np

from concourse import bacc, bass, mybir
import concourse.tile as tile
from concourse.bass_utils import run_bass_kernel_spmd

LAST_RESULTS = []

F32 = mybir.dt.float32
F16 = mybir.dt.float16
I16 = mybir.dt.int16

N_CORES = 8
W = 32            # nodes per super-window (one-hot width)
NQ = 4            # dst quarters
EPC = 128         # edge slots per chunk (partition dim)
SPG = 8           # super-windows per group
CPG = SPG * NQ    # chunks per group
EPG = CPG * EPC   # edge slots per group (4096)
OPG = SPG * W     # output node slots per group (256)


# --------------------------------------------------------------------------
# host-side layout preprocessing
# --------------------------------------------------------------------------

def _prep_core(src_s, dst_s, n_lo, n_hi, qr):
    """Pack one core's src-sorted edges into quarter-window chunks.

    Returns per-core arrays (no group padding; caller pads to uniform G):
      n_win, wbases[n_win], spans[n_win],
      e_slot (slot index per edge within G*EPG), e_idx (int16 dst local id),
      e_loc (src offset within window).
    """
    npc = n_hi - n_lo
    lo = np.searchsorted(src_s, n_lo, side="left")
    hi = np.searchsorted(src_s, n_hi, side="left")
    s_loc = (src_s[lo:hi] - n_lo).astype(np.int64)
    d = dst_s[lo:hi].astype(np.int64)
    q = d // qr

    cnt = np.bincount(s_loc * NQ + q, minlength=npc * NQ).reshape(npc, NQ)
    assert cnt.max() <= EPC, "node quarter-degree exceeds one chunk"

    wbases = [0]
    cur = cnt[0].astype(np.int64).copy()
    wid = np.empty(npc, np.int32)
    wid[0] = 0
    for n in range(1, npc):
        c = cnt[n]
        if (n - wbases[-1] >= W) or np.any(cur + c > EPC):
            wbases.append(n)
            cur = c.astype(np.int64).copy()
        else:
            cur += c
        wid[n] = len(wbases) - 1
    wbases = np.asarray(wbases, np.int64)
    n_win = len(wbases)
    spans = np.empty(n_win, np.int64)
    spans[:-1] = wbases[1:] - wbases[:-1]
    spans[-1] = npc - wbases[-1]

    # chunk id per edge, then stable sort so each chunk's edges are contiguous
    e_w = wid[s_loc]
    e_ch = e_w.astype(np.int64) * NQ + q
    order = np.argsort(e_ch, kind="stable")
    e_ch = e_ch[order]
    e_d = d[order]
    e_q = q[order]
    e_loc = (s_loc - wbases[e_w])[order]

    n_ch = n_win * NQ
    ch_cnt = np.bincount(e_ch, minlength=n_ch)
    ch_start = np.concatenate([[0], np.cumsum(ch_cnt)[:-1]])
    rank = np.arange(len(e_ch)) - ch_start[e_ch]

    # slot within [G, CPG, EPC]: group g = w//SPG, kk = q*SPG + (w%SPG)
    w_of = e_ch // NQ
    q_of = e_ch % NQ
    g = w_of // SPG
    kk = q_of * SPG + (w_of % SPG)
    e_slot = (g * CPG + kk) * EPC + rank

    e_idx = (e_d - e_q * qr).astype(np.int16)
    return n_win, wbases, spans, e_slot, e_idx, e_loc.astype(np.float16)


def _wrap16(arr2d):
    """[n, 16*k] idx array -> dma_gather wrapped layout [16, k] per row set.

    arr2d: [rows, num_idxs]; returns [rows, 16, num_idxs//16] with
    out[r, p, f] = arr2d[r, f*16 + p].
    """
    r, n = arr2d.shape
    return arr2d.reshape(r, n // 16, 16).transpose(0, 2, 1)


def _prep_edges(src, dst, n_nodes, qr):
    npc = n_nodes // N_CORES
    order = np.argsort(src, kind="stable")
    src_s = src[order]
    dst_s = dst[order]

    cores = []
    for c in range(N_CORES):
        cores.append(_prep_core(src_s, dst_s, c * npc, (c + 1) * npc, qr))
    G = max((cr[0] + SPG - 1) // SPG for cr in cores)

    metas, locms, m0Ts, cmaps = [], [], [], []
    for c in range(N_CORES):
        n_win, wbases, spans, e_slot, e_idx, e_loc = cores[c]

        xidx = np.zeros(G * EPG, np.int16)
        xidx[e_slot] = e_idx
        locv = np.full(G * EPG, -1.0, np.float16)
        locv[e_slot] = e_loc
        locq = locv.reshape(G, CPG, EPC)

        # m0T one-hot [G, W, CPG*EPC]: m0T[g, w, kk*EPC+p] = (loc(kk,p)==w)
        oh = (locq.reshape(G, CPG, 1, EPC)
              == np.arange(W, dtype=np.float16).reshape(1, 1, W, 1)
              ).astype(np.float16)              # [G, CPG, W, EPC]
        m0T = oh.transpose(0, 2, 1, 3).reshape(G, W, CPG * EPC).copy()

        # device loc layout [G, 128, CPG]
        locm = locq.transpose(0, 2, 1).copy()

        # per-window columns: pcT gather idx (int16 local node id) + colmap
        s_all = np.arange(G * SPG)
        wb = np.zeros(G * SPG, np.int64)
        sp = np.zeros(G * SPG, np.int64)
        wb[:n_win] = wbases
        sp[:n_win] = spans
        ww = np.arange(W)
        colnode = wb[:, None] + ww[None, :]          # [G*SPG, W]
        valid = ww[None, :] < sp[:, None]
        pidx_val = np.where(valid, colnode, 0).astype(np.int16)
        cmap = np.where(valid, colnode + c * npc, -1).astype(np.int64)

        # pcT idx ordering: j = s*128 + w (window s on partitions 0:W of
        # chunk s; partitions W:128 pad)
        pidx = np.zeros((G, SPG * EPC), np.int16)
        pv = pidx_val.reshape(G, SPG, W)
        pidx.reshape(G, SPG, EPC)[:, :, 0:W] = pv

        # meta int16 [G, 128, 320]: cols 0:256 = 4x wrapped xidx, 256:320 pcT
        meta = np.zeros((G, 128, 320), np.int16)
        xw = _wrap16(xidx.reshape(G * NQ, SPG * EPC)).reshape(G, NQ, 16, 64)
        for qq in range(NQ):
            meta[:, 0:16, qq * 64:(qq + 1) * 64] = xw[:, qq]
        meta[:, 0:16, 256:320] = _wrap16(pidx)
        meta[:, 16:32, :] = meta[:, 0:16, :]

        metas.append(meta)
        locms.append(locm)
        m0Ts.append(m0T)
        # output colmap in (s, w) order = s*32 + w
        cmaps.append(cmap.reshape(G, OPG))
    return dict(metas=metas, locms=locms, m0Ts=m0Ts, cmaps=cmaps, G=G)


# --------------------------------------------------------------------------
# launch 1: P = x @ A   (distributed over node slabs, batched by 4 tiles)
# --------------------------------------------------------------------------

def _build_l1(nt4):
    """xt: [128, nt4*512] f16 (x-slab transposed), amat: [128, 8] f16
    -> pout: [nt4*4, 128, 8] f32"""
    nc = bacc.Bacc(None)
    xt = nc.declare_dram_parameter("xt", [128, nt4 * 512], F16, isOutput=False)
    amat = nc.declare_dram_parameter("amat", [128, 8], F16, isOutput=False)
    pout = nc.declare_dram_parameter("pout", [nt4 * 4, 128, 8], F32,
                                     isOutput=True)

    with tile.TileContext(nc) as tc:
        with (
            tc.tile_pool(name="sb", bufs=3) as sb,
            tc.tile_pool(name="cst", bufs=1) as cst,
            tc.tile_pool(name="ps", bufs=2, space="PSUM") as ps,
        ):
            a_sb = cst.tile([128, 8], F16)
            nc.sync.dma_start(out=a_sb[:], in_=amat[:, :])
            dummy_ps = ps.tile([1, 1], F32, tag="dummy")
            nc.tensor.matmul(out=dummy_ps[:], lhsT=a_sb[:1, :1], rhs=a_sb[:1, :1],
                             start=True, stop=True)
            for t in range(nt4):
                xt_sb = sb.tile([128, 4, 128], F16, tag="xt")
                nc.sync.dma_start(out=xt_sb[:],
                                  in_=xt[:, t * 512:(t + 1) * 512])
                pp = ps.tile([128, 32], F32)
                for j in range(4):
                    nc.tensor.matmul(out=pp[:, j * 8:(j + 1) * 8],
                                     lhsT=xt_sb[:, j, :], rhs=a_sb[:],
                                     start=True, stop=True)
                p_sb = sb.tile([128, 4, 8], F32, tag="p")
                nc.vector.tensor_copy(out=p_sb[:], in_=pp[:].rearrange(
                    "p (j e) -> p j e", j=4, e=8))
                nc.sync.dma_start(
                    out=pout[t * 4:(t + 1) * 4].rearrange("j p e -> p j e"),
                    in_=p_sb[:])
    nc.compile()
    return nc


# --------------------------------------------------------------------------
# launch 2: the main edge-parallel kernel
# --------------------------------------------------------------------------

def _build_l2(G, qr, npc):
    nc = bacc.Bacc(None)
    qtabs = [nc.declare_dram_parameter(f"q{i}", [qr, 256], F16, isOutput=False)
             for i in range(NQ)]
    pcore = nc.declare_dram_parameter("pcore", [npc, 128], F16, isOutput=False)
    meta = nc.declare_dram_parameter("meta", [G, 128, 320], I16, isOutput=False)
    locmp = nc.declare_dram_parameter("locm", [G, 128, CPG], F16,
                                      isOutput=False)
    m0Tp = nc.declare_dram_parameter("m0T", [G, W, CPG * EPC], F16,
                                     isOutput=False)
    iotac = nc.declare_dram_parameter("iotac", [128, W], F16, isOutput=False)
    selc = nc.declare_dram_parameter("selc", [4, 512], F16, isOutput=False)
    wcol = nc.declare_dram_parameter("wcol", [128, 4], F32, isOutput=False)
    out = nc.declare_dram_parameter("out", [4, G, 128, OPG], F16,
                                    isOutput=True)

    with tile.TileContext(nc) as tc:
        with (
            tc.tile_pool(name="cst", bufs=1) as cst,
            tc.tile_pool(name="idx", bufs=3) as idxp,
            tc.tile_pool(name="gat", bufs=2) as gat,
            tc.tile_pool(name="mm", bufs=2) as mm,
            tc.tile_pool(name="epi", bufs=2) as epi,
            tc.tile_pool(name="outp", bufs=3) as outp,
            tc.tile_pool(name="psl", bufs=1, space="PSUM") as pslp,
            tc.tile_pool(name="agg", bufs=2, space="PSUM") as aggpool,
            tc.tile_pool(name="rs", bufs=1, space="PSUM") as rspool,
            tc.tile_pool(name="bc", bufs=1, space="PSUM") as bcpool,
        ):
            iota_sb = cst.tile([128, W], F16)
            nc.sync.dma_start(out=iota_sb[:], in_=iotac[:, :])
            sel_sb = cst.tile([4, 512], F16)
            nc.sync.dma_start(out=sel_sb[:], in_=selc[:, :])
            w_sb = cst.tile([128, 4], F32)
            nc.sync.dma_start(out=w_sb[:], in_=wcol[:, :])

            for g in range(G):
                meta_sb = idxp.tile([128, 320], I16, tag="meta")
                nc.sync.dma_start(out=meta_sb[:], in_=meta[g, :, :])
                loc_sb = idxp.tile([128, CPG], F16, tag="loc")
                nc.sync.dma_start(out=loc_sb[:], in_=locmp[g, :, :])
                m0T_sb = idxp.tile([W, CPG * EPC], F16, tag="m0T")
                nc.sync.dma_start(out=m0T_sb[:], in_=m0Tp[g, :, :])

                # ---- gathers: one per dst quarter + one for column P rows
                xg = gat.tile([128, CPG, 256], F16, tag="xg")
                for q in range(NQ):
                    nc.gpsimd.dma_gather(
                        xg[:, q * SPG:(q + 1) * SPG, :],
                        qtabs[q][:, :],
                        meta_sb[:, q * 64:(q + 1) * 64],
                        SPG * EPC, SPG * EPC, 256)
                pcT = gat.tile([128, SPG, 128], F16, tag="pcT")
                nc.gpsimd.dma_gather(
                    pcT[:, :, :], pcore[:, :], meta_sb[:, 256:320],
                    SPG * EPC, SPG * EPC, 128)

                # ---- one-hot m0 [128, CPG, W]
                m0 = mm.tile([128, CPG, W], F16, tag="m0")
                nc.vector.tensor_tensor(
                    out=m0[:],
                    in0=loc_sb[:, :, None].broadcast_to([128, CPG, W]),
                    in1=iota_sb[:, None, :].broadcast_to([128, CPG, W]),
                    op=mybir.AluOpType.is_equal)

                # ---- p_src per edge slot: P_slot = m0T^T . pcols
                psl = pslp.tile([128, CPG * 4], F32, tag="psl")
                for kk in range(CPG):
                    s = kk % SPG
                    nc.tensor.matmul(
                        out=psl[:, kk * 4:(kk + 1) * 4],
                        lhsT=m0T_sb[:, kk * EPC:(kk + 1) * EPC],
                        rhs=pcT[0:W, s, 0:4],
                        start=True, stop=True)

                # ---- per-edge factors on ACT
                asl = mm.tile([128, CPG, 4], F16, tag="asl")
                nc.scalar.activation(out=asl[:].rearrange("p c i -> p (c i)"),
                                     in_=psl[:],
                                     func=mybir.ActivationFunctionType.Exp,
                                     scale=-1.0)
                csl = mm.tile([128, CPG, 4], F16, tag="csl")
                nc.scalar.activation(out=csl[:].rearrange("p c i -> p (c i)"),
                                     in_=psl[:],
                                     func=mybir.ActivationFunctionType.Exp,
                                     scale=-0.2)
                b16 = mm.tile([128, CPG, 4], F16, tag="b16")
                nc.scalar.activation(out=b16[:], in_=xg[:, :, 132:136],
                                     func=mybir.ActivationFunctionType.Exp,
                                     scale=-1.0)
                d16 = mm.tile([128, CPG, 4], F16, tag="d16")
                nc.scalar.activation(out=d16[:], in_=xg[:, :, 132:136],
                                     func=mybir.ActivationFunctionType.Exp,
                                     scale=-0.2)

                # ---- e = min(A*B, C*D)
                ab = mm.tile([128, CPG, 4], F16, tag="ab")
                nc.vector.tensor_tensor(out=ab[:], in0=asl[:], in1=b16[:],
                                        op=mybir.AluOpType.mult)
                cd = mm.tile([128, CPG, 4], F16, tag="cd")
                nc.vector.tensor_tensor(out=cd[:], in0=csl[:], in1=d16[:],
                                        op=mybir.AluOpType.mult)
                e4 = mm.tile([128, CPG, 4], F16, tag="e4")
                nc.vector.tensor_tensor(out=e4[:], in0=ab[:], in1=cd[:],
                                        op=mybir.AluOpType.min)

                # ---- weighted one-hot
                mall = mm.tile([128, CPG, 4, W], F16, tag="mall")
                nc.vector.tensor_tensor(
                    out=mall[:],
                    in0=m0[:, :, None, :].broadcast_to([128, CPG, 4, W]),
                    in1=e4[:, :, :, None].broadcast_to([128, CPG, 4, W]),
                    op=mybir.AluOpType.mult)

                # ---- segment sums; 4 quarter-chunks accumulate per window
                aggp = aggpool.tile([128, SPG * 4 * W], F32, tag="agg")
                rsp = rspool.tile([4, SPG * W], F32, tag="rs")
                for s in range(SPG):
                    for q in range(NQ):
                        kk = q * SPG + s
                        nc.tensor.matmul(
                            out=aggp[:, s * 128:(s + 1) * 128],
                            lhsT=xg[:, kk, 0:128], rhs=mall[:, kk, :, :],
                            start=(q == 0), stop=(q == 3))
                    for q in range(NQ):
                        kk = q * SPG + s
                        nc.tensor.matmul(
                            out=rsp[:, s * W:(s + 1) * W],
                            lhsT=e4[:, kk, :], rhs=m0[:, kk, :],
                            start=(q == 0), stop=(q == 3))

                # ---- reciprocal of rowsums (clamp pad zeros)
                rsc = epi.tile([4, OPG], F32, tag="rsc")
                nc.vector.tensor_scalar(out=rsc[:], in0=rsp[:], scalar1=3e-5,
                                        scalar2=None, op0=mybir.AluOpType.max)
                rsi = epi.tile([4, OPG], F32, tag="rsi")
                nc.vector.reciprocal(out=rsi[:], in_=rsc[:])
                rsi16 = epi.tile([4, OPG], F16, tag="rsi16")
                nc.vector.tensor_copy(out=rsi16[:], in_=rsi[:])

                # ---- epilogue: out = w * agg * (1/rowsum), f16
                agg4 = aggp[:].rearrange("p (s i w) -> p s i w", s=SPG, i=4,
                                         w=W)
                for i in range(4):
                    bc = bcpool.tile([128, OPG], F32, tag="bc")
                    nc.tensor.matmul(out=bc[:],
                                     lhsT=sel_sb[:, i * 128:(i + 1) * 128],
                                     rhs=rsi16[:], start=True, stop=True)
                    rinv = epi.tile([128, OPG], F32, tag="rinv")
                    nc.scalar.activation(out=rinv[:], in_=bc[:],
                                         func=mybir.ActivationFunctionType.Copy)
                    oh = outp.tile([128, OPG], F16, tag="oh")
                    nc.vector.scalar_tensor_tensor(
                        out=oh[:].rearrange("p (s w) -> p s w", s=SPG, w=W),
                        in0=agg4[:, :, i, :],
                        scalar=w_sb[:, i:i + 1],
                        in1=rinv[:].rearrange("p (s w) -> p s w", s=SPG, w=W),
                        op0=mybir.AluOpType.mult, op1=mybir.AluOpType.mult)
                    nc.sync.dma_start(out=out[i, g, :, :], in_=oh[:])
    nc.compile()
    return nc


# --------------------------------------------------------------------------
# entry point
# --------------------------------------------------------------------------

def kernel(x, w, attn, edge, _n_cores=N_CORES):
    x = np.asarray(x, dtype=np.float32)
    w = np.asarray(w, dtype=np.float32)
    attn = np.asarray(attn, dtype=np.float32)
    edge = np.asarray(edge)

    n_nodes, d = x.shape
    n_heads = w.shape[0]
    assert d == 128 and n_heads == 4
    qr = n_nodes // NQ
    npc = n_nodes // N_CORES

    src = edge[0].astype(np.int64)
    dst = edge[1].astype(np.int64)

    # fold parameters: A[:, i] = w_i * a_src_i ; A[:, 4+i] = w_i * a_dst_i
    amat = np.zeros((128, 8), dtype=np.float32)
    for i in range(n_heads):
        amat[:, i] = w[i, 0, :] * attn[i, :d, 0]
        amat[:, 4 + i] = w[i, 0, :] * attn[i, d:, 0]

    # ---------------- launch 1: P = x @ A (node slabs)
    nt = (npc + 127) // 128
    nt4 = (nt + 3) // 4
    nc1 = _build_l1(nt4)
    amat16 = amat.astype(np.float16)
    in_maps1 = []
    for c in range(N_CORES):
        sl = x[c * npc:(c + 1) * npc]
        if sl.shape[0] < nt4 * 512:
            sl = np.concatenate(
                [sl, np.zeros((nt4 * 512 - sl.shape[0], d), np.float32)])
        in_maps1.append({"xt": np.ascontiguousarray(sl.T).astype(np.float16),
                         "amat": amat16})
    trace = bool(int(os.environ.get("GAT_TRACE", "0")))
    tkw = dict(trace=True, trace_cores=list(range(N_CORES))) if trace else {}

    def _run(nc, maps):
        try:
            return run_bass_kernel_spmd(nc, maps, list(range(N_CORES)), **tkw)
        except Exception:
            if not tkw:
                raise
            return run_bass_kernel_spmd(nc, maps, list(range(N_CORES)))

    r1 = _run(nc1, in_maps1)
    ptab = np.concatenate(
        [r1.results[c]["pout"].reshape(-1, 8)[:npc] for c in range(N_CORES)],
        axis=0)

    # ---------------- host layout prep
    prep = _prep_edges(src, dst, n_nodes, qr)
    G = prep["G"]

    # ---------------- launch 2
    nc2 = _build_l2(G, qr, npc)
    t512 = np.zeros((n_nodes, 256), dtype=np.float16)
    t512[:, 0:128] = x.astype(np.float16)
    t512[:, 128:136] = ptab.astype(np.float16)
    qtabs = [np.ascontiguousarray(t512[i * qr:(i + 1) * qr])
             for i in range(NQ)]
    iota_c = np.broadcast_to(np.arange(W, dtype=np.float16), (128, W)).copy()
    sel_c = np.zeros((4, 512), dtype=np.float16)
    for i in range(4):
        sel_c[i, i * 128:(i + 1) * 128] = 1.0
    wcol = np.ascontiguousarray(w[:, 0, :].T)  # [128, 4]
    in_maps2 = []
    for c in range(N_CORES):
        pcore = np.zeros((npc, 128), dtype=np.float16)
        pcore[:, 0:4] = ptab[c * npc:(c + 1) * npc, 0:4].astype(np.float16)
        m = {"pcore": pcore,
             "meta": prep["metas"][c], "locm": prep["locms"][c],
             "m0T": prep["m0Ts"][c],
             "iotac": iota_c, "selc": sel_c, "wcol": wcol}
        for i in range(NQ):
            m[f"q{i}"] = qtabs[i]
        in_maps2.append(m)
    r2 = _run(nc2, in_maps2)
    LAST_RESULTS.clear()
    LAST_RESULTS.extend([r1, r2])

    # ---------------- unshard: scatter window columns to node rows
    out_full = np.zeros((n_heads, n_nodes, d), dtype=np.float32)
    for c in range(N_CORES):
        slab = r2.results[c]["out"]      # [4, G, 128, OPG] f16
        cm = prep["cmaps"][c].reshape(-1)
        arr = slab.transpose(0, 1, 3, 2).reshape(n_heads, G * OPG, d)
        valid = cm >= 0
        out_full[:, cm[valid], :] = arr[:, valid, :].astype(np.float32)
    return out_full


if __name__ == "__main__":
    pass


# revision 8
# speedup vs baseline: 1.3840x; 1.0497x over previous
"""Multi-head graph attention (GAT-style message passing) on 8 Trainium2 cores.

Math (per head i, diag transform):
    h        = x * w[i]                      # [N, d]
    p_src    = h @ a[:d],  p_dst = h @ a[d:] # [N]
    s_e      = p_src[src_e] + p_dst[dst_e]   # per edge
    e_e      = exp(-leaky_relu(s_e, 0.2))
    out[i,n] = (sum_{e: src=n} e_e * h[dst_e]) / (sum_{e: src=n} e_e)

Key identities:
  - w[i] commutes with the segment sum, so the x[dst] gather is shared by all
    4 heads and w[i] is applied at the very end.
  - exp(-leaky_relu(s)) = min(exp(-s), exp(-0.2 s)) and s factors into
    p_src[src] + p_dst[dst], so the per-edge weight is
    min(A[src]*B[dst], C[src]*D[dst]) with A,C per source column and B,D
    carried by the gathered destination row.

Layout ("quarter-window" scheme):
  - Edges are sorted by src and partitioned across 8 cores by src range.
  - A super-window = up to 32 consecutive src nodes whose per-dst-quarter
    edge counts each fit in 128.  Each window emits 4 chunks (one per dst
    quarter, 128 edge slots each); dst indices within a chunk are local to
    that quarter (< 25000), so they fit the int16 indices of the gpsimd
    dma_gather custom op.  One dma_gather per (group, quarter) replaces the
    128-row indirect DMAs that dominated the old kernel (994ns+ each on the
    Pool engine).
  - A group = 8 windows = 32 chunks = 4096 edge slots.  The 4 chunks of a
    window accumulate into the same PSUM block (matmul start/stop), so
    per-window rowsums and aggregates are complete on device.
  - p_src per edge slot comes from a host-built transposed one-hot (m0T)
    matmul against the window's 32 column P-values (gathered per group from
    a per-core P table, int16-local again).
  - Output is written f16, [4, G, 128, 8*32] per core, host scatters rows.
"""

import os

import numpy as np

from concourse import bacc, bass, mybir
import concourse.tile as tile
from concourse.bass_utils import run_bass_kernel_spmd

LAST_RESULTS = []

F32 = mybir.dt.float32
F16 = mybir.dt.float16
I16 = mybir.dt.int16

N_CORES = 8
W = 32            # nodes per super-window (one-hot width)
NQ = 4            # dst quarters
EPC = 128         # edge slots per chunk (partition dim)
SPG = 8           # super-windows per group
CPG = SPG * NQ    # chunks per group
EPG = CPG * EPC   # edge slots per group (4096)
OPG = SPG * W     # output node slots per group (256)


# --------------------------------------------------------------------------
# host-side layout preprocessing
# --------------------------------------------------------------------------

def _prep_core(src_s, dst_s, n_lo, n_hi, qr):
    """Pack one core's src-sorted edges into quarter-window chunks.

    Returns per-core arrays (no group padding; caller pads to uniform G):
      n_win, wbases[n_win], spans[n_win],
      e_slot (slot index per edge within G*EPG), e_idx (int16 dst local id),
      e_loc (src offset within window).
    """
    npc = n_hi - n_lo
    lo = np.searchsorted(src_s, n_lo, side="left")
    hi = np.searchsorted(src_s, n_hi, side="left")
    s_loc = (src_s[lo:hi] - n_lo).astype(np.int64)
    d = dst_s[lo:hi].astype(np.int64)
    q = d // qr

    cnt = np.bincount(s_loc * NQ + q, minlength=npc * NQ).reshape(npc, NQ)
    assert cnt.max() <= EPC, "node quarter-degree exceeds one chunk"

    wbases = [0]
    cur = cnt[0].astype(np.int64).copy()
    wid = np.empty(npc, np.int32)
    wid[0] = 0
    for n in range(1, npc):
        c = cnt[n]
        if (n - wbases[-1] >= W) or np.any(cur + c > EPC):
            wbases.append(n)
            cur = c.astype(np.int64).copy()
        else:
            cur += c
        wid[n] = len(wbases) - 1
    wbases = np.asarray(wbases, np.int64)
    n_win = len(wbases)
    spans = np.empty(n_win, np.int64)
    spans[:-1] = wbases[1:] - wbases[:-1]
    spans[-1] = npc - wbases[-1]

    # chunk id per edge, then stable sort so each chunk's edges are contiguous
    e_w = wid[s_loc]
    e_ch = e_w.astype(np.int64) * NQ + q
    order = np.argsort(e_ch, kind="stable")
    e_ch = e_ch[order]
    e_d = d[order]
    e_q = q[order]
    e_loc = (s_loc - wbases[e_w])[order]

    n_ch = n_win * NQ
    ch_cnt = np.bincount(e_ch, minlength=n_ch)
    ch_start = np.concatenate([[0], np.cumsum(ch_cnt)[:-1]])
    rank = np.arange(len(e_ch)) - ch_start[e_ch]

    # slot within [G, CPG, EPC]: group g = w//SPG, kk = q*SPG + (w%SPG)
    w_of = e_ch // NQ
    q_of = e_ch % NQ
    g = w_of // SPG
    kk = q_of * SPG + (w_of % SPG)
    e_slot = (g * CPG + kk) * EPC + rank

    e_idx = (e_d - e_q * qr).astype(np.int16)
    return n_win, wbases, spans, e_slot, e_idx, e_loc.astype(np.float16)


def _wrap16(arr2d):
    """[n, 16*k] idx array -> dma_gather wrapped layout [16, k] per row set.

    arr2d: [rows, num_idxs]; returns [rows, 16, num_idxs//16] with
    out[r, p, f] = arr2d[r, f*16 + p].
    """
    r, n = arr2d.shape
    return arr2d.reshape(r, n // 16, 16).transpose(0, 2, 1)


def _prep_edges(src, dst, n_nodes, qr):
    npc = n_nodes // N_CORES
    order = np.argsort(src, kind="stable")
    src_s = src[order]
    dst_s = dst[order]

    cores = []
    for c in range(N_CORES):
        cores.append(_prep_core(src_s, dst_s, c * npc, (c + 1) * npc, qr))
    G = max((cr[0] + SPG - 1) // SPG for cr in cores)

    metas, locms, m0Ts, cmaps = [], [], [], []
    for c in range(N_CORES):
        n_win, wbases, spans, e_slot, e_idx, e_loc = cores[c]

        xidx = np.zeros(G * EPG, np.int16)
        xidx[e_slot] = e_idx
        locv = np.full(G * EPG, -1.0, np.float16)
        locv[e_slot] = e_loc
        locq = locv.reshape(G, CPG, EPC)

        # m0T one-hot [G, W, CPG*EPC]: m0T[g, w, kk*EPC+p] = (loc(kk,p)==w)
        oh = (locq.reshape(G, CPG, 1, EPC)
              == np.arange(W, dtype=np.float16).reshape(1, 1, W, 1)
              ).astype(np.float16)              # [G, CPG, W, EPC]
        m0T = oh.transpose(0, 2, 1, 3).reshape(G, W, CPG * EPC).copy()

        # device loc layout [G, 128, CPG]
        locm = locq.transpose(0, 2, 1).copy()

        # per-window columns: pcT gather idx (int16 local node id) + colmap
        s_all = np.arange(G * SPG)
        wb = np.zeros(G * SPG, np.int64)
        sp = np.zeros(G * SPG, np.int64)
        wb[:n_win] = wbases
        sp[:n_win] = spans
        ww = np.arange(W)
        colnode = wb[:, None] + ww[None, :]          # [G*SPG, W]
        valid = ww[None, :] < sp[:, None]
        cmap = np.where(valid, colnode + c * npc, -1).astype(np.int64)

        # meta int16 [G, 128, 256]: cols = 4x wrapped xidx
        meta = np.zeros((G, 128, 256), np.int16)
        xw = _wrap16(xidx.reshape(G * NQ, SPG * EPC)).reshape(G, NQ, 16, 64)
        for qq in range(NQ):
            meta[:, 0:16, qq * 64:(qq + 1) * 64] = xw[:, qq]
        meta[:, 16:32, :] = meta[:, 0:16, :]

        metas.append(meta)
        locms.append(locm)
        m0Ts.append(m0T)
        # output colmap in (s, w) order = s*32 + w
        cmaps.append(cmap.reshape(G, OPG))
    return dict(metas=metas, locms=locms, m0Ts=m0Ts, cmaps=cmaps, G=G)


# --------------------------------------------------------------------------
# launch 1: P = x @ A   (distributed over node slabs, batched by 4 tiles)
# --------------------------------------------------------------------------

def _build_l1(nt4):
    """xt: [128, nt4*512] f16 (x-slab transposed), amat: [128, 8] f16
    -> pout: [nt4*4, 128, 8] f32"""
    nc = bacc.Bacc(None)
    xt = nc.declare_dram_parameter("xt", [128, nt4 * 512], F16, isOutput=False)
    amat = nc.declare_dram_parameter("amat", [128, 8], F16, isOutput=False)
    pout = nc.declare_dram_parameter("pout", [nt4 * 4, 128, 8], F32,
                                     isOutput=True)

    with tile.TileContext(nc) as tc:
        with (
            tc.tile_pool(name="sb", bufs=3) as sb,
            tc.tile_pool(name="cst", bufs=1) as cst,
            tc.tile_pool(name="ps", bufs=2, space="PSUM") as ps,
        ):
            a_sb = cst.tile([128, 8], F16)
            nc.sync.dma_start(out=a_sb[:], in_=amat[:, :])
            dummy_ps = ps.tile([1, 1], F32, tag="dummy")
            nc.tensor.matmul(out=dummy_ps[:], lhsT=a_sb[:1, :1], rhs=a_sb[:1, :1],
                             start=True, stop=True)
            for t in range(nt4):
                xt_sb = sb.tile([128, 4, 128], F16, tag="xt")
                nc.sync.dma_start(out=xt_sb[:],
                                  in_=xt[:, t * 512:(t + 1) * 512])
                pp = ps.tile([128, 32], F32)
                for j in range(4):
                    nc.tensor.matmul(out=pp[:, j * 8:(j + 1) * 8],
                                     lhsT=xt_sb[:, j, :], rhs=a_sb[:],
                                     start=True, stop=True)
                p_sb = sb.tile([128, 4, 8], F32, tag="p")
                nc.vector.tensor_copy(out=p_sb[:], in_=pp[:].rearrange(
                    "p (j e) -> p j e", j=4, e=8))
                nc.sync.dma_start(
                    out=pout[t * 4:(t + 1) * 4].rearrange("j p e -> p j e"),
                    in_=p_sb[:])
    nc.compile()
    return nc


# --------------------------------------------------------------------------
# launch 2: the main edge-parallel kernel
# --------------------------------------------------------------------------

def _build_l2(G, qr, npc):
    nc = bacc.Bacc(None)
    qtabs = [nc.declare_dram_parameter(f"q{i}", [qr, 256], F16, isOutput=False)
             for i in range(NQ)]
    pcolmp = nc.declare_dram_parameter("pcolm", [G, W, SPG * 4], F16,
                                       isOutput=False)
    meta = nc.declare_dram_parameter("meta", [G, 128, 256], I16, isOutput=False)
    locmp = nc.declare_dram_parameter("locm", [G, 128, CPG], F16,
                                      isOutput=False)
    m0Tp = nc.declare_dram_parameter("m0T", [G, W, CPG * EPC], F16,
                                     isOutput=False)
    iotac = nc.declare_dram_parameter("iotac", [128, W], F16, isOutput=False)
    selc = nc.declare_dram_parameter("selc", [4, 512], F16, isOutput=False)
    wcol = nc.declare_dram_parameter("wcol", [128, 4], F32, isOutput=False)
    out = nc.declare_dram_parameter("out", [4, G, 128, OPG], F16,
                                    isOutput=True)

    with tile.TileContext(nc) as tc:
        with (
            tc.tile_pool(name="cst", bufs=1) as cst,
            tc.tile_pool(name="idx", bufs=3) as idxp,
            tc.tile_pool(name="gat", bufs=3) as gat,
            tc.tile_pool(name="mm", bufs=2) as mm,
            tc.tile_pool(name="epi", bufs=2) as epi,
            tc.tile_pool(name="outp", bufs=3) as outp,
            tc.tile_pool(name="psl", bufs=1, space="PSUM") as pslp,
            tc.tile_pool(name="agg", bufs=2, space="PSUM") as aggpool,
            tc.tile_pool(name="rs", bufs=1, space="PSUM") as rspool,
            tc.tile_pool(name="bc", bufs=1, space="PSUM") as bcpool,
        ):
            iota_sb = cst.tile([128, W], F16)
            nc.sync.dma_start(out=iota_sb[:], in_=iotac[:, :])
            sel_sb = cst.tile([4, 512], F16)
            nc.sync.dma_start(out=sel_sb[:], in_=selc[:, :])
            w_sb = cst.tile([128, 4], F32)
            nc.sync.dma_start(out=w_sb[:], in_=wcol[:, :])

            for g in range(G):
                meta_sb = idxp.tile([128, 256], I16, tag="meta")
                nc.sync.dma_start(out=meta_sb[:], in_=meta[g, :, :])
                loc_sb = idxp.tile([128, CPG], F16, tag="loc")
                nc.sync.dma_start(out=loc_sb[:], in_=locmp[g, :, :])
                m0T_sb = idxp.tile([W, CPG * EPC], F16, tag="m0T")
                nc.sync.dma_start(out=m0T_sb[:], in_=m0Tp[g, :, :])

                # ---- gathers: one per dst quarter + one for column P rows
                xg = gat.tile([128, CPG, 256], F16, tag="xg")
                for q in range(NQ):
                    nc.gpsimd.dma_gather(
                        xg[:, q * SPG:(q + 1) * SPG, :],
                        qtabs[q][:, :],
                        meta_sb[:, q * 64:(q + 1) * 64],
                        SPG * EPC, SPG * EPC, 256)
                pcol_sb = idxp.tile([W, SPG * 4], F16, tag="pcol")
                nc.sync.dma_start(out=pcol_sb[:], in_=pcolmp[g, :, :])

                # ---- one-hot m0 [128, CPG, W]
                m0 = mm.tile([128, CPG, W], F16, tag="m0")
                nc.vector.tensor_tensor(
                    out=m0[:],
                    in0=loc_sb[:, :, None].broadcast_to([128, CPG, W]),
                    in1=iota_sb[:, None, :].broadcast_to([128, CPG, W]),
                    op=mybir.AluOpType.is_equal)

                # ---- p_src per edge slot: P_slot = m0T^T . pcols
                psl = pslp.tile([128, CPG * 4], F32, tag="psl")
                for kk in range(CPG):
                    s = kk % SPG
                    nc.tensor.matmul(
                        out=psl[:, kk * 4:(kk + 1) * 4],
                        lhsT=m0T_sb[:, kk * EPC:(kk + 1) * EPC],
                        rhs=pcol_sb[:, s * 4:(s + 1) * 4],
                        start=True, stop=True)

                # ---- per-edge factors on ACT
                asl = mm.tile([128, CPG, 4], F16, tag="asl")
                nc.scalar.activation(out=asl[:].rearrange("p c i -> p (c i)"),
                                     in_=psl[:],
                                     func=mybir.ActivationFunctionType.Exp,
                                     scale=-1.0)
                csl = mm.tile([128, CPG, 4], F16, tag="csl")
                nc.scalar.activation(out=csl[:].rearrange("p c i -> p (c i)"),
                                     in_=psl[:],
                                     func=mybir.ActivationFunctionType.Exp,
                                     scale=-0.2)
                b16 = mm.tile([128, CPG, 4], F16, tag="b16")
                nc.scalar.activation(out=b16[:], in_=xg[:, :, 132:136],
                                     func=mybir.ActivationFunctionType.Exp,
                                     scale=-1.0)
                d16 = mm.tile([128, CPG, 4], F16, tag="d16")
                nc.scalar.activation(out=d16[:], in_=xg[:, :, 132:136],
                                     func=mybir.ActivationFunctionType.Exp,
                                     scale=-0.2)

                # ---- e = min(A*B, C*D)
                ab = mm.tile([128, CPG, 4], F16, tag="ab")
                nc.vector.tensor_tensor(out=ab[:], in0=asl[:], in1=b16[:],
                                        op=mybir.AluOpType.mult)
                cd = mm.tile([128, CPG, 4], F16, tag="cd")
                nc.vector.tensor_tensor(out=cd[:], in0=csl[:], in1=d16[:],
                                        op=mybir.AluOpType.mult)
                e4 = mm.tile([128, CPG, 4], F16, tag="e4")
                nc.vector.tensor_tensor(out=e4[:], in0=ab[:], in1=cd[:],
                                        op=mybir.AluOpType.min)

                # ---- weighted one-hot
                mall = mm.tile([128, CPG, 4, W], F16, tag="mall")
                nc.vector.tensor_tensor(
                    out=mall[:],
                    in0=m0[:, :, None, :].broadcast_to([128, CPG, 4, W]),
                    in1=e4[:, :, :, None].broadcast_to([128, CPG, 4, W]),
                    op=mybir.AluOpType.mult)

                # ---- segment sums; 4 quarter-chunks accumulate per window
                aggp = aggpool.tile([128, SPG * 4 * W], F32, tag="agg")
                rsp = rspool.tile([4, SPG * W], F32, tag="rs")
                for s in range(SPG):
                    for q in range(NQ):
                        kk = q * SPG + s
                        nc.tensor.matmul(
                            out=aggp[:, s * 128:(s + 1) * 128],
                            lhsT=xg[:, kk, 0:128], rhs=mall[:, kk, :, :],
                            start=(q == 0), stop=(q == 3))
                    for q in range(NQ):
                        kk = q * SPG + s
                        nc.tensor.matmul(
                            out=rsp[:, s * W:(s + 1) * W],
                            lhsT=e4[:, kk, :], rhs=m0[:, kk, :],
                            start=(q == 0), stop=(q == 3))

                # ---- reciprocal of rowsums (clamp pad zeros)
                rsc = epi.tile([4, OPG], F32, tag="rsc")
                nc.vector.tensor_scalar(out=rsc[:], in0=rsp[:], scalar1=3e-5,
                                        scalar2=None, op0=mybir.AluOpType.max)
                rsi16 = epi.tile([4, OPG], F16, tag="rsi16")
                with nc.allow_low_precision(reason="attention rowsum recip"):
                    nc.vector.reciprocal(out=rsi16[:], in_=rsc[:])

                # ---- epilogue: out = w * agg * (1/rowsum), f16
                agg4 = aggp[:].rearrange("p (s i w) -> p s i w", s=SPG, i=4,
                                         w=W)
                for i in range(4):
                    bc = bcpool.tile([128, OPG], F32, tag="bc")
                    nc.tensor.matmul(out=bc[:],
                                     lhsT=sel_sb[:, i * 128:(i + 1) * 128],
                                     rhs=rsi16[:], start=True, stop=True)
                    rinv = epi.tile([128, OPG], F32, tag="rinv")
                    nc.scalar.activation(out=rinv[:], in_=bc[:],
                                         func=mybir.ActivationFunctionType.Copy)
                    oh = outp.tile([128, OPG], F16, tag="oh")
                    nc.vector.scalar_tensor_tensor(
                        out=oh[:].rearrange("p (s w) -> p s w", s=SPG, w=W),
                        in0=agg4[:, :, i, :],
                        scalar=w_sb[:, i:i + 1],
                        in1=rinv[:].rearrange("p (s w) -> p s w", s=SPG, w=W),
                        op0=mybir.AluOpType.mult, op1=mybir.AluOpType.mult)
                    nc.sync.dma_start(out=out[i, g, :, :], in_=oh[:])
    nc.compile()
    return nc


# --------------------------------------------------------------------------
# entry point
# --------------------------------------------------------------------------

def kernel(x, w, attn, edge, _n_cores=N_CORES):
    x = np.asarray(x, dtype=np.float32)
    w = np.asarray(w, dtype=np.float32)
    attn = np.asarray(attn, dtype=np.float32)
    edge = np.asarray(edge)

    n_nodes, d = x.shape
    n_heads = w.shape[0]
    assert d == 128 and n_heads == 4
    qr = n_nodes // NQ
    npc = n_nodes // N_CORES

    src = edge[0].astype(np.int64)
    dst = edge[1].astype(np.int64)

    # fold parameters: A[:, i] = w_i * a_src_i ; A[:, 4+i] = w_i * a_dst_i
    amat = np.zeros((128, 8), dtype=np.float32)
    for i in range(n_heads):
        amat[:, i] = w[i, 0, :] * attn[i, :d, 0]
        amat[:, 4 + i] = w[i, 0, :] * attn[i, d:, 0]

    # ---------------- launch 1: P = x @ A (node slabs)
    nt = (npc + 127) // 128
    nt4 = (nt + 3) // 4
    nc1 = _build_l1(nt4)
    amat16 = amat.astype(np.float16)
    in_maps1 = []
    for c in range(N_CORES):
        sl = x[c * npc:(c + 1) * npc]
        if sl.shape[0] < nt4 * 512:
            sl = np.concatenate(
                [sl, np.zeros((nt4 * 512 - sl.shape[0], d), np.float32)])
        in_maps1.append({"xt": np.ascontiguousarray(sl.T).astype(np.float16),
                         "amat": amat16})
    trace = bool(int(os.environ.get("GAT_TRACE", "0")))
    tkw = dict(trace=True, trace_cores=list(range(N_CORES))) if trace else {}

    def _run(nc, maps):
        try:
            return run_bass_kernel_spmd(nc, maps, list(range(N_CORES)), **tkw)
        except Exception:
            if not tkw:
                raise
            return run_bass_kernel_spmd(nc, maps, list(range(N_CORES)))

    r1 = _run(nc1, in_maps1)
    ptab = np.concatenate(
        [r1.results[c]["pout"].reshape(-1, 8)[:npc] for c in range(N_CORES)],
        axis=0)

    # ---------------- host layout prep
    prep = _prep_edges(src, dst, n_nodes, qr)
    G = prep["G"]

    # ---------------- launch 2
    nc2 = _build_l2(G, qr, npc)
    t512 = np.zeros((n_nodes, 256), dtype=np.float16)
    t512[:, 0:128] = x.astype(np.float16)
    t512[:, 128:136] = ptab.astype(np.float16)
    qtabs = [np.ascontiguousarray(t512[i * qr:(i + 1) * qr])
             for i in range(NQ)]
    iota_c = np.broadcast_to(np.arange(W, dtype=np.float16), (128, W)).copy()
    sel_c = np.zeros((4, 512), dtype=np.float16)
    for i in range(4):
        sel_c[i, i * 128:(i + 1) * 128] = 1.0
    wcol = np.ascontiguousarray(w[:, 0, :].T)  # [128, 4]
    in_maps2 = []
    for c in range(N_CORES):
        cmap = prep["cmaps"][c].reshape(G, SPG, W)
        pc4 = ptab[np.maximum(cmap, 0), 0:4].astype(np.float16)
        pc4[cmap < 0] = 0
        pcolm = np.ascontiguousarray(
            pc4.transpose(0, 2, 1, 3).reshape(G, W, SPG * 4))
        m = {"pcolm": pcolm,
             "meta": prep["metas"][c], "locm": prep["locms"][c],
             "m0T": prep["m0Ts"][c],
             "iotac": iota_c, "selc": sel_c, "wcol": wcol}
        for i in range(NQ):
            m[f"q{i}"] = qtabs[i]
        in_maps2.append(m)
    r2 = _run(nc2, in_maps2)
    LAST_RESULTS.clear()
    LAST_RESULTS.extend([r1, r2])

    # ---------------- unshard: scatter window columns to node rows
    out_full = np.zeros((n_heads, n_nodes, d), dtype=np.float32)
    for c in range(N_CORES):
        slab = r2.results[c]["out"]      # [4, G, 128, OPG] f16
        cm = prep["cmaps"][c].reshape(-1)
        arr = slab.transpose(0, 1, 3, 2).reshape(n_heads, G * OPG, d)
        valid = cm >= 0
        out_full[:, cm[valid], :] = arr[:, valid, :].astype(np.float32)
    return out_full


if __name__ == "__main__":
    pass


# revision 11
# speedup vs baseline: 1.5944x; 1.1521x over previous
"""Multi-head graph attention (GAT-style message passing) on 8 Trainium2 cores.

Math (per head i, diag transform):
    h        = x * w[i]                      # [N, d]
    p_src    = h @ a[:d],  p_dst = h @ a[d:] # [N]
    s_e      = p_src[src_e] + p_dst[dst_e]   # per edge
    e_e      = exp(-leaky_relu(s_e, 0.2))
    out[i,n] = (sum_{e: src=n} e_e * h[dst_e]) / (sum_{e: src=n} e_e)

Key identities:
  - w[i] commutes with the segment sum, so the x[dst] gather is shared by all
    4 heads and w[i] is applied at the very end.
  - exp(-leaky_relu(s)) = min(exp(-s), exp(-0.2 s)) and s factors into
    p_src[src] + p_dst[dst], so the per-edge weight is
    min(A[src]*B[dst], C[src]*D[dst]) with A,C per source column and B,D
    carried by the gathered destination row.

Layout ("quarter-window" scheme):
  - Edges are sorted by src and partitioned across 8 cores by src range.
  - A super-window = up to 32 consecutive src nodes whose per-dst-quarter
    edge counts each fit in 128.  Each window emits 4 chunks (one per dst
    quarter, 128 edge slots each); dst indices within a chunk are local to
    that quarter (< 25000), so they fit the int16 indices of the gpsimd
    dma_gather custom op.  One dma_gather per (group, quarter) replaces the
    128-row indirect DMAs that dominated the old kernel (994ns+ each on the
    Pool engine).
  - A group = 8 windows = 32 chunks = 4096 edge slots.  The 4 chunks of a
    window accumulate into the same PSUM block (matmul start/stop), so
    per-window rowsums and aggregates are complete on device.
  - p_src per edge slot comes from a host-built transposed one-hot (m0T)
    matmul against the window's 32 column P-values (gathered per group from
    a per-core P table, int16-local again).
  - Output is written f16, [4, G, 128, 8*32] per core, host scatters rows.
"""

import os

import numpy as np

from concourse import bacc, bass, mybir
import concourse.tile as tile
from concourse.bass_utils import run_bass_kernel_spmd

LAST_RESULTS = []

F32 = mybir.dt.float32
F16 = mybir.dt.float16
I16 = mybir.dt.int16

N_CORES = 8
W = 32            # nodes per super-window (one-hot width)
NQ = 4            # dst quarters
EPC = 128         # edge slots per chunk (partition dim)
SPG = 8           # super-windows per group
CPG = SPG * NQ    # chunks per group
EPG = CPG * EPC   # edge slots per group (4096)
OPG = SPG * W     # output node slots per group (256)


# --------------------------------------------------------------------------
# host-side layout preprocessing
# --------------------------------------------------------------------------

def _prep_core(src_s, dst_s, n_lo, n_hi, qr):
    """Pack one core's src-sorted edges into quarter-window chunks.

    Returns per-core arrays (no group padding; caller pads to uniform G):
      n_win, wbases[n_win], spans[n_win],
      e_slot (slot index per edge within G*EPG), e_idx (int16 dst local id),
      e_loc (src offset within window).
    """
    npc = n_hi - n_lo
    lo = np.searchsorted(src_s, n_lo, side="left")
    hi = np.searchsorted(src_s, n_hi, side="left")
    s_loc = (src_s[lo:hi] - n_lo).astype(np.int64)
    d = dst_s[lo:hi].astype(np.int64)
    q = d // qr

    cnt = np.bincount(s_loc * NQ + q, minlength=npc * NQ).reshape(npc, NQ)
    assert cnt.max() <= EPC, "node quarter-degree exceeds one chunk"

    wbases = [0]
    cur = cnt[0].astype(np.int64).copy()
    wid = np.empty(npc, np.int32)
    wid[0] = 0
    for n in range(1, npc):
        c = cnt[n]
        if (n - wbases[-1] >= W) or np.any(cur + c > EPC):
            wbases.append(n)
            cur = c.astype(np.int64).copy()
        else:
            cur += c
        wid[n] = len(wbases) - 1
    wbases = np.asarray(wbases, np.int64)
    n_win = len(wbases)
    spans = np.empty(n_win, np.int64)
    spans[:-1] = wbases[1:] - wbases[:-1]
    spans[-1] = npc - wbases[-1]

    # chunk id per edge, then stable sort so each chunk's edges are contiguous
    e_w = wid[s_loc]
    e_ch = e_w.astype(np.int64) * NQ + q
    order = np.argsort(e_ch, kind="stable")
    e_ch = e_ch[order]
    e_d = d[order]
    e_q = q[order]
    e_loc = (s_loc - wbases[e_w])[order]

    n_ch = n_win * NQ
    ch_cnt = np.bincount(e_ch, minlength=n_ch)
    ch_start = np.concatenate([[0], np.cumsum(ch_cnt)[:-1]])
    rank = np.arange(len(e_ch)) - ch_start[e_ch]

    # slot within [G, CPG, EPC]: group g = w//SPG, kk = q*SPG + (w%SPG)
    w_of = e_ch // NQ
    q_of = e_ch % NQ
    g = w_of // SPG
    kk = q_of * SPG + (w_of % SPG)
    e_slot = (g * CPG + kk) * EPC + rank

    e_idx = (e_d - e_q * qr).astype(np.int16)
    return n_win, wbases, spans, e_slot, e_idx, e_loc.astype(np.float16)


def _wrap16(arr2d):
    """[n, 16*k] idx array -> dma_gather wrapped layout [16, k] per row set.

    arr2d: [rows, num_idxs]; returns [rows, 16, num_idxs//16] with
    out[r, p, f] = arr2d[r, f*16 + p].
    """
    r, n = arr2d.shape
    return arr2d.reshape(r, n // 16, 16).transpose(0, 2, 1)


def _prep_edges(src, dst, n_nodes, qr):
    npc = n_nodes // N_CORES
    order = np.argsort(src, kind="stable")
    src_s = src[order]
    dst_s = dst[order]

    cores = []
    for c in range(N_CORES):
        cores.append(_prep_core(src_s, dst_s, c * npc, (c + 1) * npc, qr))
    G = max((cr[0] + SPG - 1) // SPG for cr in cores)

    metas, locms, m0Ts, cmaps = [], [], [], []
    for c in range(N_CORES):
        n_win, wbases, spans, e_slot, e_idx, e_loc = cores[c]

        xidx = np.zeros(G * EPG, np.int16)
        xidx[e_slot] = e_idx
        locv = np.full(G * EPG, -1.0, np.float16)
        locv[e_slot] = e_loc
        locq = locv.reshape(G, CPG, EPC)

        # m0T one-hot [G, W, CPG*EPC]: m0T[g, w, kk*EPC+p] = (loc(kk,p)==w)
        oh = (locq.reshape(G, CPG, 1, EPC)
              == np.arange(W, dtype=np.float16).reshape(1, 1, W, 1)
              ).astype(np.float16)              # [G, CPG, W, EPC]
        m0T = oh.transpose(0, 2, 1, 3).reshape(G, W, CPG * EPC).copy()

        # device loc layout [G, 128, CPG]
        locm = locq.transpose(0, 2, 1).copy()

        # per-window columns: pcT gather idx (int16 local node id) + colmap
        s_all = np.arange(G * SPG)
        wb = np.zeros(G * SPG, np.int64)
        sp = np.zeros(G * SPG, np.int64)
        wb[:n_win] = wbases
        sp[:n_win] = spans
        ww = np.arange(W)
        colnode = wb[:, None] + ww[None, :]          # [G*SPG, W]
        valid = ww[None, :] < sp[:, None]
        cmap = np.where(valid, colnode + c * npc, -1).astype(np.int64)

        # meta int16 [G, 128, NQ*QC]: cols = 4x wrapped xidx
        qc = SPG * EPC // 16
        meta = np.zeros((G, 128, NQ * qc), np.int16)
        xw = _wrap16(xidx.reshape(G * NQ, SPG * EPC)).reshape(G, NQ, 16, qc)
        for qq in range(NQ):
            meta[:, 0:16, qq * qc:(qq + 1) * qc] = xw[:, qq]
        meta[:, 16:32, :] = meta[:, 0:16, :]

        metas.append(meta)
        locms.append(locm)
        m0Ts.append(m0T)
        # output colmap in (s, w) order = s*32 + w
        cmaps.append(cmap.reshape(G, OPG))
    return dict(metas=metas, locms=locms, m0Ts=m0Ts, cmaps=cmaps, G=G)


# --------------------------------------------------------------------------
# launch 1: P = x @ A   (distributed over node slabs, batched by 4 tiles)
# --------------------------------------------------------------------------

def _build_l1(nt4):
    """xt: [128, nt4*512] f16 (x-slab transposed), amat: [128, 8] f16
    -> pout: [nt4*4, 128, 8] f32"""
    nc = bacc.Bacc(None)
    xt = nc.declare_dram_parameter("xt", [128, nt4 * 512], F16, isOutput=False)
    amat = nc.declare_dram_parameter("amat", [128, 8], F16, isOutput=False)
    pout = nc.declare_dram_parameter("pout", [nt4 * 4, 128, 8], F32,
                                     isOutput=True)

    with tile.TileContext(nc) as tc:
        with (
            tc.tile_pool(name="sb", bufs=3) as sb,
            tc.tile_pool(name="cst", bufs=1) as cst,
            tc.tile_pool(name="ps", bufs=2, space="PSUM") as ps,
        ):
            a_sb = cst.tile([128, 8], F16)
            nc.sync.dma_start(out=a_sb[:], in_=amat[:, :])
            dummy_ps = ps.tile([1, 1], F32, tag="dummy")
            nc.tensor.matmul(out=dummy_ps[:], lhsT=a_sb[:1, :1], rhs=a_sb[:1, :1],
                             start=True, stop=True)
            for t in range(nt4):
                xt_sb = sb.tile([128, 4, 128], F16, tag="xt")
                nc.sync.dma_start(out=xt_sb[:],
                                  in_=xt[:, t * 512:(t + 1) * 512])
                pp = ps.tile([128, 32], F32)
                for j in range(4):
                    nc.tensor.matmul(out=pp[:, j * 8:(j + 1) * 8],
                                     lhsT=xt_sb[:, j, :], rhs=a_sb[:],
                                     start=True, stop=True)
                p_sb = sb.tile([128, 4, 8], F32, tag="p")
                nc.vector.tensor_copy(out=p_sb[:], in_=pp[:].rearrange(
                    "p (j e) -> p j e", j=4, e=8))
                nc.sync.dma_start(
                    out=pout[t * 4:(t + 1) * 4].rearrange("j p e -> p j e"),
                    in_=p_sb[:])
    nc.compile()
    return nc


# --------------------------------------------------------------------------
# launch 2: the main edge-parallel kernel
# --------------------------------------------------------------------------

def _build_l2(G, qr, npc):
    nc = bacc.Bacc(None)
    qtabs = [nc.declare_dram_parameter(f"q{i}", [qr, 256], F16, isOutput=False)
             for i in range(NQ)]
    pcolmp = nc.declare_dram_parameter("pcolm", [G, W, SPG * 4], F16,
                                       isOutput=False)
    meta = nc.declare_dram_parameter("meta", [G, 128, NQ * SPG * EPC // 16], I16, isOutput=False)
    locmp = nc.declare_dram_parameter("locm", [G, 128, CPG], F16,
                                      isOutput=False)
    m0Tp = nc.declare_dram_parameter("m0T", [G, W, CPG * EPC], F16,
                                     isOutput=False)
    iotac = nc.declare_dram_parameter("iotac", [128, W], F16, isOutput=False)
    selc = nc.declare_dram_parameter("selc", [4, 512], F16, isOutput=False)
    wcol = nc.declare_dram_parameter("wcol", [128, 4], F32, isOutput=False)
    out = nc.declare_dram_parameter("out", [4, G, 128, OPG], F16,
                                    isOutput=True)

    with tile.TileContext(nc) as tc:
        with (
            tc.tile_pool(name="cst", bufs=1) as cst,
            tc.tile_pool(name="idx", bufs=3) as idxp,
            tc.tile_pool(name="gat", bufs=2) as gat,
            tc.tile_pool(name="mm", bufs=2) as mm,
            tc.tile_pool(name="epi", bufs=2) as epi,
            tc.tile_pool(name="outp", bufs=3) as outp,
            tc.tile_pool(name="psl", bufs=1, space="PSUM") as pslp,
            tc.tile_pool(name="agg", bufs=2, space="PSUM") as aggpool,
            tc.tile_pool(name="rs", bufs=1, space="PSUM") as rspool,
            tc.tile_pool(name="bc", bufs=1, space="PSUM") as bcpool,
        ):
            iota_sb = cst.tile([128, W], F16)
            nc.sync.dma_start(out=iota_sb[:], in_=iotac[:, :])
            sel_sb = cst.tile([4, 512], F16)
            nc.sync.dma_start(out=sel_sb[:], in_=selc[:, :])
            w_sb = cst.tile([128, 4], F32)
            nc.sync.dma_start(out=w_sb[:], in_=wcol[:, :])

            for g in range(G):
                meta_sb = idxp.tile([128, NQ * SPG * EPC // 16], I16, tag="meta")
                nc.sync.dma_start(out=meta_sb[:], in_=meta[g, :, :])
                loc_sb = idxp.tile([128, CPG], F16, tag="loc")
                nc.sync.dma_start(out=loc_sb[:], in_=locmp[g, :, :])
                m0T_sb = idxp.tile([W, CPG * EPC], F16, tag="m0T")
                nc.sync.dma_start(out=m0T_sb[:], in_=m0Tp[g, :, :])

                # ---- gathers: one per dst quarter + one for column P rows
                xg = gat.tile([128, CPG, 256], F16, tag="xg")
                for q in range(NQ):
                    nc.gpsimd.dma_gather(
                        xg[:, q * SPG:(q + 1) * SPG, :],
                        qtabs[q][:, :],
                        meta_sb[:, q * (SPG * EPC // 16):
                                (q + 1) * (SPG * EPC // 16)],
                        SPG * EPC, SPG * EPC, 256)
                pcol_sb = idxp.tile([W, SPG * 4], F16, tag="pcol")
                nc.sync.dma_start(out=pcol_sb[:], in_=pcolmp[g, :, :])

                # ---- one-hot m0 [128, CPG, W]
                m0 = mm.tile([128, CPG, W], F16, tag="m0")
                nc.vector.tensor_tensor(
                    out=m0[:],
                    in0=loc_sb[:, :, None].broadcast_to([128, CPG, W]),
                    in1=iota_sb[:, None, :].broadcast_to([128, CPG, W]),
                    op=mybir.AluOpType.is_equal)

                # ---- p_src per edge slot: P_slot = m0T^T . pcols
                psl = pslp.tile([128, CPG * 4], F32, tag="psl")
                for kk in range(CPG):
                    s = kk % SPG
                    nc.tensor.matmul(
                        out=psl[:, kk * 4:(kk + 1) * 4],
                        lhsT=m0T_sb[:, kk * EPC:(kk + 1) * EPC],
                        rhs=pcol_sb[:, s * 4:(s + 1) * 4],
                        start=True, stop=True)

                # ---- per-edge factors on ACT
                asl = mm.tile([128, CPG, 4], F16, tag="asl")
                nc.scalar.activation(out=asl[:].rearrange("p c i -> p (c i)"),
                                     in_=psl[:],
                                     func=mybir.ActivationFunctionType.Exp,
                                     scale=-1.0)
                csl = mm.tile([128, CPG, 4], F16, tag="csl")
                nc.scalar.activation(out=csl[:].rearrange("p c i -> p (c i)"),
                                     in_=psl[:],
                                     func=mybir.ActivationFunctionType.Exp,
                                     scale=-0.2)
                b16 = mm.tile([128, CPG, 4], F16, tag="b16")
                nc.scalar.activation(out=b16[:], in_=xg[:, :, 132:136],
                                     func=mybir.ActivationFunctionType.Exp,
                                     scale=-1.0)
                d16 = mm.tile([128, CPG, 4], F16, tag="d16")
                nc.scalar.activation(out=d16[:], in_=xg[:, :, 132:136],
                                     func=mybir.ActivationFunctionType.Exp,
                                     scale=-0.2)

                # ---- e = min(A*B, C*D)
                ab = mm.tile([128, CPG, 4], F16, tag="ab")
                nc.vector.tensor_tensor(out=ab[:], in0=asl[:], in1=b16[:],
                                        op=mybir.AluOpType.mult)
                cd = mm.tile([128, CPG, 4], F16, tag="cd")
                nc.vector.tensor_tensor(out=cd[:], in0=csl[:], in1=d16[:],
                                        op=mybir.AluOpType.mult)
                e4 = mm.tile([128, CPG, 4], F16, tag="e4")
                nc.vector.tensor_tensor(out=e4[:], in0=ab[:], in1=cd[:],
                                        op=mybir.AluOpType.min)

                # ---- weighted one-hot
                mall = mm.tile([128, CPG, 4, W], F16, tag="mall")
                nc.vector.tensor_tensor(
                    out=mall[:],
                    in0=m0[:, :, None, :].broadcast_to([128, CPG, 4, W]),
                    in1=e4[:, :, :, None].broadcast_to([128, CPG, 4, W]),
                    op=mybir.AluOpType.mult)

                # ---- segment sums; 4 quarter-chunks accumulate per window
                aggp = aggpool.tile([128, SPG * 4 * W], F32, tag="agg")
                rsp = rspool.tile([4, SPG * W], F32, tag="rs")
                for s in range(SPG):
                    for q in range(NQ):
                        kk = q * SPG + s
                        nc.tensor.matmul(
                            out=aggp[:, s * 128:(s + 1) * 128],
                            lhsT=xg[:, kk, 0:128], rhs=mall[:, kk, :, :],
                            start=(q == 0), stop=(q == 3))
                    for q in range(NQ):
                        kk = q * SPG + s
                        nc.tensor.matmul(
                            out=rsp[:, s * W:(s + 1) * W],
                            lhsT=e4[:, kk, :], rhs=m0[:, kk, :],
                            start=(q == 0), stop=(q == 3))

                # ---- reciprocal of rowsums (clamp pad zeros)
                rsc = epi.tile([4, OPG], F32, tag="rsc")
                nc.vector.tensor_scalar(out=rsc[:], in0=rsp[:], scalar1=3e-5,
                                        scalar2=None, op0=mybir.AluOpType.max)
                rsi16 = epi.tile([4, OPG], F16, tag="rsi16")
                with nc.allow_low_precision(reason="attention rowsum recip"):
                    nc.vector.reciprocal(out=rsi16[:], in_=rsc[:])

                # ---- epilogue: out = w * agg * (1/rowsum), f16
                agg4 = aggp[:].rearrange("p (s i w) -> p s i w", s=SPG, i=4,
                                         w=W)
                for i in range(4):
                    bc = bcpool.tile([128, OPG], F32, tag="bc")
                    nc.tensor.matmul(out=bc[:],
                                     lhsT=sel_sb[:, i * 128:(i + 1) * 128],
                                     rhs=rsi16[:], start=True, stop=True)
                    rinv = epi.tile([128, OPG], F32, tag="rinv")
                    nc.scalar.activation(out=rinv[:], in_=bc[:],
                                         func=mybir.ActivationFunctionType.Copy)
                    oh = outp.tile([128, OPG], F16, tag="oh")
                    nc.vector.scalar_tensor_tensor(
                        out=oh[:].rearrange("p (s w) -> p s w", s=SPG, w=W),
                        in0=agg4[:, :, i, :],
                        scalar=w_sb[:, i:i + 1],
                        in1=rinv[:].rearrange("p (s w) -> p s w", s=SPG, w=W),
                        op0=mybir.AluOpType.mult, op1=mybir.AluOpType.mult)
                    nc.sync.dma_start(out=out[i, g, :, :], in_=oh[:])
    nc.compile()
    return nc


# --------------------------------------------------------------------------
# entry point
# --------------------------------------------------------------------------

def kernel(x, w, attn, edge, _n_cores=N_CORES):
    x = np.asarray(x, dtype=np.float32)
    w = np.asarray(w, dtype=np.float32)
    attn = np.asarray(attn, dtype=np.float32)
    edge = np.asarray(edge)

    n_nodes, d = x.shape
    n_heads = w.shape[0]
    assert d == 128 and n_heads == 4
    qr = n_nodes // NQ
    npc = n_nodes // N_CORES

    src = edge[0].astype(np.int64)
    dst = edge[1].astype(np.int64)

    # fold parameters: A[:, i] = w_i * a_src_i ; A[:, 4+i] = w_i * a_dst_i
    amat = np.zeros((128, 8), dtype=np.float32)
    for i in range(n_heads):
        amat[:, i] = w[i, 0, :] * attn[i, :d, 0]
        amat[:, 4 + i] = w[i, 0, :] * attn[i, d:, 0]

    # ---------------- launch 1: P = x @ A (node slabs)
    nt = (npc + 127) // 128
    nt4 = (nt + 3) // 4
    nc1 = _build_l1(nt4)
    amat16 = amat.astype(np.float16)
    in_maps1 = []
    for c in range(N_CORES):
        sl = x[c * npc:(c + 1) * npc]
        if sl.shape[0] < nt4 * 512:
            sl = np.concatenate(
                [sl, np.zeros((nt4 * 512 - sl.shape[0], d), np.float32)])
        in_maps1.append({"xt": np.ascontiguousarray(sl.T).astype(np.float16),
                         "amat": amat16})
    trace = bool(int(os.environ.get("GAT_TRACE", "0")))
    tkw = dict(trace=True, trace_cores=list(range(N_CORES))) if trace else {}

    def _run(nc, maps):
        try:
            return run_bass_kernel_spmd(nc, maps, list(range(N_CORES)), **tkw)
        except Exception:
            if not tkw:
                raise
            return run_bass_kernel_spmd(nc, maps, list(range(N_CORES)))

    r1 = _run(nc1, in_maps1)
    ptab = np.concatenate(
        [r1.results[c]["pout"].reshape(-1, 8)[:npc] for c in range(N_CORES)],
        axis=0)

    # ---------------- host layout prep
    prep = _prep_edges(src, dst, n_nodes, qr)
    G = prep["G"]

    # ---------------- launch 2
    nc2 = _build_l2(G, qr, npc)
    t512 = np.zeros((n_nodes, 256), dtype=np.float16)
    t512[:, 0:128] = x.astype(np.float16)
    t512[:, 128:136] = ptab.astype(np.float16)
    qtabs = [np.ascontiguousarray(t512[i * qr:(i + 1) * qr])
             for i in range(NQ)]
    iota_c = np.broadcast_to(np.arange(W, dtype=np.float16), (128, W)).copy()
    sel_c = np.zeros((4, 512), dtype=np.float16)
    for i in range(4):
        sel_c[i, i * 128:(i + 1) * 128] = 1.0
    wcol = np.ascontiguousarray(w[:, 0, :].T)  # [128, 4]
    in_maps2 = []
    for c in range(N_CORES):
        cmap = prep["cmaps"][c].reshape(G, SPG, W)
        pc4 = ptab[np.maximum(cmap, 0), 0:4].astype(np.float16)
        pc4[cmap < 0] = 0
        pcolm = np.ascontiguousarray(
            pc4.transpose(0, 2, 1, 3).reshape(G, W, SPG * 4))
        m = {"pcolm": pcolm,
             "meta": prep["metas"][c], "locm": prep["locms"][c],
             "m0T": prep["m0Ts"][c],
             "iotac": iota_c, "selc": sel_c, "wcol": wcol}
        for i in range(NQ):
            m[f"q{i}"] = qtabs[i]
        in_maps2.append(m)
    r2 = _run(nc2, in_maps2)
    LAST_RESULTS.clear()
    LAST_RESULTS.extend([r1, r2])

    # ---------------- unshard: scatter window columns to node rows
    out_full = np.zeros((n_heads, n_nodes, d), dtype=np.float32)
    for c in range(N_CORES):
        slab = r2.results[c]["out"]      # [4, G, 128, OPG] f16
        cm = prep["cmaps"][c].reshape(-1)
        arr = slab.transpose(0, 1, 3, 2).reshape(n_heads, G * OPG, d)
        valid = cm >= 0
        out_full[:, cm[valid], :] = arr[:, valid, :].astype(np.float32)
    return out_full


if __name__ == "__main__":
    pass
